# revision 1
# baseline (speedup 1.0000x reference)
"""Trainium2 Bass kernel for GNN NodeBlock (segment_sum + MLP), 8-core SPMD.

Strategy (node-sharded, two-path aggregation, transpose-free):
  - Shard the 100k nodes across 8 cores (12500 each). Host sorts edges by
    receiver.
  - Dense path: each node gets 16 padded edge slots (~90% of edges). The
    host lays each pair of slot-planes out as a [128, 512] tile whose
    partition axis is (slot-pair-member, hi/lo, feature) and free axis is
    the 512 nodes of a supergroup. A constant stationary [I32;I32;I32;I32]
    then makes each matmul compute psum[f, n] += hi+lo of both slots — the
    PSUM accumulates the feature-major aggregate directly (no one-hot, no
    transpose, no weight reloads).
  - Overflow path: edges beyond a node's 16th go through a one-hot matmul
    (is_equal vs iota on DVE): psum[:, window] += edges_hi^T @ onehot and
    edges_lo^T @ onehot.
  - The [32, 512] aggregate is drained once per supergroup as bf16 hi
    (ACT cast-copy) + bf16 lo (DVE subtract), duplicated into a K-stacked
    [hi; hi; lo] layout by SBUF-to-SBUF DMA, and consumed by K-stacked bf16
    MLP matmuls (exact to ~2^-16) with weight hi/lo splits baked in on the
    host. The globals term is folded into b1 on the host.
  - No collectives: cores own disjoint node ranges; host concatenates.
"""

import os

import numpy as np
import ml_dtypes

import concourse.bacc as bacc
import concourse.bass as bass
import concourse.mybir as mybir
import concourse.tile as tile
from concourse.bass_utils import run_bass_kernel_spmd

BF16 = ml_dtypes.bfloat16

N_NODES = 100000
N_CORES = 8
NPC = N_NODES // N_CORES  # 12500 nodes per core
P = 128
SG = 512  # nodes per supergroup (4 windows of 128)
NSG = -(-NPC // SG)  # 25 supergroups per core
WPS = SG // P  # 4 windows per supergroup
G = NSG * WPS  # 100 windows incl. dummies (98 real)
KD = 16  # dense slots per node
QD = KD // 2  # slot pairs -> dense matmuls per supergroup
D = 32

_prog_cache = {}


def _split_hi_lo(x):
    hi = x.astype(BF16)
    lo = (x - hi.astype(np.float32)).astype(BF16)
    return hi, lo


def _host_prep(node_attr, edge_index, edge_attr, global_attr, W1, b1, W2, b2):
    E = edge_attr.shape[0]
    r = np.ascontiguousarray(edge_index[1]).astype(np.int64)

    order = np.argsort(r, kind="stable")
    r_s = r[order]
    deg = np.bincount(r, minlength=N_NODES)
    starts = np.zeros(N_NODES, dtype=np.int64)
    np.cumsum(deg[:-1], out=starts[1:])
    k = np.arange(E, dtype=np.int64) - starts[r_s]  # rank within node

    ea = np.ascontiguousarray(edge_attr, dtype=np.float32)
    hi, lo = _split_hi_lo(ea)
    hilo = np.concatenate([hi, lo], axis=1)[order]  # (E, 64) in sorted order

    core = r_s // NPC
    local = r_s - core * NPC
    sg = local // SG
    n512 = local % SG
    j = n512 // P
    w_in = (local % P).astype(np.float32)

    dense = k < KD
    # (c, sg, q, sp, n, chan) -> partitions (sp, chan), free (q, n)
    TA = np.zeros((N_CORES, NSG, QD, 2, SG, 2 * D), dtype=BF16)
    TA[core[dense], sg[dense], k[dense] // 2, k[dense] % 2, n512[dense]] = hilo[dense]
    arr_A = np.ascontiguousarray(TA.transpose(0, 1, 3, 5, 2, 4)).reshape(
        N_CORES, NSG, P, QD * SG
    )

    # overflow: bucket per (core, window); windows 98,99 are dummies
    ov = ~dense
    wkey = (core * G + local // P).astype(np.int64)[ov]
    cnt = np.bincount(wkey, minlength=N_CORES * G)
    m_l = max(1, int(-(-cnt.max() // P)))
    starts2 = np.zeros(N_CORES * G, dtype=np.int64)
    np.cumsum(cnt[:-1], out=starts2[1:])
    seq = np.zeros(E, dtype=np.int64)
    seq[ov] = np.arange(int(ov.sum()), dtype=np.int64)
    slot2 = seq[ov] - starts2[wkey]
    t2 = slot2 // P
    p2 = slot2 % P

    arr_B = np.zeros((N_CORES, NSG, P, WPS, m_l, 2 * D), dtype=BF16)
    arr_B[core[ov], sg[ov], p2, j[ov], t2] = hilo[ov]

    widx = np.full((N_CORES * G, m_l * P), -1.0, dtype=np.float32)
    widx[wkey, slot2] = w_in[ov]
    # (c, w=sg*4+jj, t, p) -> (c, p, sg, jj, t)
    idx_arr = np.ascontiguousarray(
        widx.reshape(N_CORES, NSG, WPS, m_l, P).transpose(0, 4, 1, 2, 3)
    ).reshape(N_CORES, P, NSG * WPS * m_l).astype(BF16)

    edges_in = np.concatenate(
        [
            arr_A,
            arr_B.reshape(N_CORES, NSG, P, WPS * m_l * 2 * D),
        ],
        axis=3,
    )

    # stationary for the dense path: [I32; I32; I32; I32]
    ident4 = np.ascontiguousarray(np.tile(np.eye(D, dtype=BF16), (4, 1)))

    nodeT = np.ascontiguousarray(
        node_attr.astype(np.float32).reshape(N_CORES, NPC, D).transpose(0, 2, 1)
    )
    nhi, nlo = _split_hi_lo(nodeT)
    node_stack = np.concatenate([nhi, nhi, nlo], axis=1)  # (8, 96, NPC)

    g0 = global_attr.astype(np.float32).reshape(1, D)
    W1 = W1.astype(np.float32)
    b1p = (b1.astype(np.float32) + (g0 @ W1[2 * D :]).reshape(-1)).reshape(D, 1)
    w1a_hi, w1a_lo = _split_hi_lo(W1[:D])
    w1b_hi, w1b_lo = _split_hi_lo(W1[D : 2 * D])
    w1n = np.ascontiguousarray(np.concatenate([w1a_hi, w1a_lo, w1a_hi], axis=0))
    w1g = np.ascontiguousarray(np.concatenate([w1b_hi, w1b_lo, w1b_hi], axis=0))
    w2_hi, w2_lo = _split_hi_lo(W2.astype(np.float32))
    w2_hi = np.ascontiguousarray(w2_hi)
    w2_lo = np.ascontiguousarray(w2_lo)
    b2 = b2.astype(np.float32).reshape(D, 1)

    in_maps = []
    for c in range(N_CORES):
        in_maps.append(
            {
                "edges": edges_in[c],
                "idx": idx_arr[c],
                "ident4": ident4,
                "nodeS": node_stack[c],
                "w1n": w1n,
                "w1g": w1g,
                "w2h": w2_hi,
                "w2l": w2_lo,
                "b1p": b1p,
                "b2": b2,
            }
        )
    return in_maps, m_l


def _build_program(m_l):
    if m_l in _prog_cache:
        return _prog_cache[m_l]

    f32 = mybir.dt.float32
    bf16 = mybir.dt.bfloat16
    nc = bacc.Bacc(
        "TRN2", target_bir_lowering=False, debug=False, num_devices=N_CORES
    )

    A_ELEMS = QD * SG  # 4096
    B_ELEMS = WPS * m_l * 2 * D
    NT = WPS * m_l  # overflow tiles per supergroup

    edges_d = nc.dram_tensor(
        "edges", [NSG, P, A_ELEMS + B_ELEMS], bf16, kind="ExternalInput"
    )
    idx_d = nc.dram_tensor("idx", [P, NSG * NT], bf16, kind="ExternalInput")
    ident4_d = nc.dram_tensor("ident4", [P, D], bf16, kind="ExternalInput")
    nodeS_d = nc.dram_tensor("nodeS", [3 * D, NPC], bf16, kind="ExternalInput")
    w1n_d = nc.dram_tensor("w1n", [3 * D, D], bf16, kind="ExternalInput")
    w1g_d = nc.dram_tensor("w1g", [3 * D, D], bf16, kind="ExternalInput")
    w2h_d = nc.dram_tensor("w2h", [D, D], bf16, kind="ExternalInput")
    w2l_d = nc.dram_tensor("w2l", [D, D], bf16, kind="ExternalInput")
    b1p_d = nc.dram_tensor("b1p", [D, 1], f32, kind="ExternalInput")
    b2_d = nc.dram_tensor("b2", [D, 1], f32, kind="ExternalInput")
    outT_d = nc.dram_tensor("outT", [D, NPC], f32, kind="ExternalOutput")

    with tile.TileContext(nc) as tc:
        with (
            tc.tile_pool(name="const", bufs=1) as cpool,
            tc.tile_pool(name="edges", bufs=3) as epool,
            tc.tile_pool(name="oh", bufs=3) as opool,
            tc.tile_pool(name="mlp", bufs=2) as mpool,
            tc.tile_pool(name="psA", bufs=4, space="PSUM") as pspool,
            tc.tile_pool(name="psM", bufs=2, space="PSUM") as pmpool,
        ):
            # constants
            iota32 = cpool.tile([P, NT, P], mybir.dt.int32)
            nc.gpsimd.iota(
                iota32[:], pattern=[[0, NT], [1, P]], base=0, channel_multiplier=0
            )
            iotab = cpool.tile([P, NT, P], bf16)
            nc.vector.tensor_copy(out=iotab[:], in_=iota32[:])

            ident4_sb = cpool.tile([P, D], bf16)
            nc.sync.dma_start(out=ident4_sb[:], in_=ident4_d.ap())
            idx_all = cpool.tile([P, NSG * NT], bf16)
            nc.sync.dma_start(out=idx_all[:], in_=idx_d.ap())
            nodeS_sb = cpool.tile([3 * D, NPC], bf16)
            nc.sync.dma_start(out=nodeS_sb[:], in_=nodeS_d.ap())
            w1n_sb = cpool.tile([3 * D, D], bf16)
            nc.sync.dma_start(out=w1n_sb[:], in_=w1n_d.ap())
            w1g_sb = cpool.tile([3 * D, D], bf16)
            nc.sync.dma_start(out=w1g_sb[:], in_=w1g_d.ap())
            w2h_sb = cpool.tile([D, D], bf16)
            nc.sync.dma_start(out=w2h_sb[:], in_=w2h_d.ap())
            w2l_sb = cpool.tile([D, D], bf16)
            nc.sync.dma_start(out=w2l_sb[:], in_=w2l_d.ap())
            b1p_sb = cpool.tile([D, 1], f32)
            nc.sync.dma_start(out=b1p_sb[:], in_=b1p_d.ap())
            b2_sb = cpool.tile([D, 1], f32)
            nc.sync.dma_start(out=b2_sb[:], in_=b2_d.ap())

            # agg stack [hi; hi; lo] built via DMA duplication per supergroup
            aggS = cpool.tile([3 * D, G * P], bf16)  # [96, 12800]
            aggL = cpool.tile([D, G * P], bf16)  # lo scratch [32, 12800]

            CH = 512
            for s in range(NSG):
                edges_t = epool.tile([P, A_ELEMS + B_ELEMS], bf16)
                nc.sync.dma_start(out=edges_t[:], in_=edges_d.ap()[s])
                oh = opool.tile([P, NT, P], bf16)
                nc.vector.tensor_tensor(
                    out=oh[:],
                    in0=iotab[:],
                    in1=idx_all[:, s * NT : (s + 1) * NT].to_broadcast([P, NT, P]),
                    op=mybir.AluOpType.is_equal,
                )
                ps = pspool.tile([D, SG], f32)
                for q in range(QD):
                    nc.tensor.matmul(
                        out=ps[:],
                        lhsT=ident4_sb[:],
                        rhs=edges_t[:, q * SG : (q + 1) * SG],
                        start=(q == 0),
                        stop=False,
                        skip_group_check=True,
                    )
                for jt in range(NT):
                    jj = jt // m_l
                    base = A_ELEMS + jt * 2 * D
                    wcols = slice(jj * P, (jj + 1) * P)
                    nc.tensor.matmul(
                        out=ps[:, wcols],
                        lhsT=edges_t[:, base : base + D],
                        rhs=oh[:, jt, :],
                        start=False,
                        stop=False,
                        skip_group_check=True,
                    )
                    nc.tensor.matmul(
                        out=ps[:, wcols],
                        lhsT=edges_t[:, base + D : base + 2 * D],
                        rhs=oh[:, jt, :],
                        start=False,
                        stop=(jt == NT - 1),
                        skip_group_check=True,
                    )
                sgc = slice(s * SG, (s + 1) * SG)
                nc.scalar.activation(
                    out=aggS[:D, sgc],
                    in_=ps[:],
                    func=mybir.ActivationFunctionType.Copy,
                )
                nc.vector.tensor_tensor(
                    out=aggL[:, sgc],
                    in0=ps[:],
                    in1=aggS[:D, sgc],
                    op=mybir.AluOpType.subtract,
                )
                nc.sync.dma_start(out=aggS[D : 2 * D, sgc], in_=aggS[:D, sgc])
                nc.sync.dma_start(out=aggS[2 * D :, sgc], in_=aggL[:, sgc])

                # MLP chunk for this supergroup's nodes
                n = min(CH, NPC - s * CH)
                cols = slice(s * CH, s * CH + n)
                ph = pmpool.tile([D, CH], f32, tag="ph")
                nc.tensor.matmul(
                    out=ph[:, :n],
                    lhsT=w1n_sb[:],
                    rhs=nodeS_sb[:, cols],
                    start=True,
                    stop=False,
                )
                nc.tensor.matmul(
                    out=ph[:, :n],
                    lhsT=w1g_sb[:],
                    rhs=aggS[:, cols],
                    start=False,
                    stop=True,
                )
                hT = mpool.tile([D, CH], f32, tag="hT")
                nc.scalar.activation(
                    out=hT[:, :n],
                    in_=ph[:, :n],
                    func=mybir.ActivationFunctionType.Relu,
                    bias=b1p_sb[:],
                    scale=1.0,
                )
                hH = mpool.tile([D, CH], bf16, tag="hH")
                nc.scalar.activation(
                    out=hH[:, :n],
                    in_=hT[:, :n],
                    func=mybir.ActivationFunctionType.Copy,
                )
                hL = mpool.tile([D, CH], bf16, tag="hL")
                nc.vector.tensor_tensor(
                    out=hL[:, :n],
                    in0=hT[:, :n],
                    in1=hH[:, :n],
                    op=mybir.AluOpType.subtract,
                )
                po = pmpool.tile([D, CH], f32, tag="po")
                nc.tensor.matmul(
                    out=po[:, :n], lhsT=w2h_sb[:], rhs=hH[:, :n], start=True, stop=False
                )
                nc.tensor.matmul(
                    out=po[:, :n],
                    lhsT=w2l_sb[:],
                    rhs=hH[:, :n],
                    start=False,
                    stop=False,
                )
                nc.tensor.matmul(
                    out=po[:, :n], lhsT=w2h_sb[:], rhs=hL[:, :n], start=False, stop=True
                )
                ot = mpool.tile([D, CH], f32, tag="ot")
                nc.vector.tensor_tensor(
                    out=ot[:, :n],
                    in0=po[:, :n],
                    in1=b2_sb[:].to_broadcast([D, n]),
                    op=mybir.AluOpType.add,
                )
                nc.sync.dma_start(out=outT_d.ap()[:, cols], in_=ot[:, :n])

    nc.finalize()
    _prog_cache[m_l] = nc
    return nc


def kernel(**inputs):
    in_maps, m_l = _host_prep(**inputs)
    nc = _build_program(m_l)
    trace = bool(os.environ.get("KERNEL_TRACE"))
    res = run_bass_kernel_spmd(nc, in_maps, list(range(N_CORES)), trace=trace)
    if trace:
        print(f"HW exec time: {res.exec_time_ns} ns")
        print(f"mean exec time: {res.mean_exec_time_ns} ns")
    out = np.empty((N_NODES, D), dtype=np.float32)
    for c in range(N_CORES):
        out[c * NPC : (c + 1) * NPC] = res.results[c]["outT"].T
    return out



# revision 2
# speedup vs baseline: 2.2133x; 2.2133x over previous
"""Trainium2 Bass kernel for GNN NodeBlock (segment_sum + MLP), 8-core SPMD.

Strategy (node-sharded, fp16, degree-balanced):
  - Host assigns nodes to (core, supergroup, window, col) by dealing them
    round-robin in decreasing order of overflow degree, so every 128-node
    window sees ~equal overflow (max <= 256 edges -> m_l=2) and every core
    ~equal edge bytes. Output is un-permuted on the host.
  - All tensor data travels as fp16 (the 2e-2 error gate leaves ~40x
    margin; measured pipeline rel err ~5e-4), halving HBM bytes and PE
    work vs an f32-exact hi/lo split.
  - Dense path: 16 fp16 slots per node, 4 slots stacked on partitions
    (4 slots x 32 feats = 128), summed into PSUM by a [I32;I32;I32;I32]
    stationary: 4 matmuls of 512 cols per supergroup.
  - Overflow path (edges past a node's 16th): one-hot matmul per
    128-edge tile (is_equal vs iota on DVE), 2 tiles per window.
  - MLP: agg is drained once to fp16, combined with a per-supergroup
    node-attr DMA into a [64, 512] tile; W1 (globals folded into b1 on
    host), ReLU via scalar ACT bias path, W2, +b2 via DVE; fp16 out.
  - No collectives: cores own disjoint node ranges; host gathers.
"""

import os

import numpy as np

import concourse.bacc as bacc
import concourse.bass as bass
import concourse.mybir as mybir
import concourse.tile as tile
from concourse.bass_utils import run_bass_kernel_spmd

F16 = np.float16

N_NODES = 100000
N_CORES = 8
P = 128
SG = 512  # nodes per supergroup (4 windows of 128)
WPS = SG // P  # 4 windows per supergroup
NSG = 25  # supergroups per core
NPC_PAD = NSG * SG  # 12800 node columns per core (125/128 used per window)
NW = N_CORES * NSG * WPS  # 800 windows
KD = 16  # dense slots per node
QD = KD // 4  # 4 slots per matmul pass -> 4 dense matmuls per supergroup
D = 32
A_ELEMS = QD * SG  # 2048 dense cols per supergroup

_prog_cache = {}


def _host_prep(node_attr, edge_index, edge_attr, global_attr, W1, b1, W2, b2):
    E = edge_attr.shape[0]
    r = np.ascontiguousarray(edge_index[1]).astype(np.int64)

    # ---- balanced node placement ----
    deg = np.bincount(r, minlength=N_NODES)
    excess = np.maximum(deg - KD, 0)
    order_n = np.argsort(-excess, kind="stable")
    win = np.empty(N_NODES, np.int64)
    win[order_n] = np.arange(N_NODES) % NW
    colw = np.empty(N_NODES, np.int64)
    colw[order_n] = np.arange(N_NODES) // NW  # 0..124
    core_of = win // (NSG * WPS)
    sg_of = (win % (NSG * WPS)) // WPS
    j_of = win % WPS
    loc_of = sg_of * SG + j_of * P + colw  # column within core

    # ---- per-edge placement (receiver-sorted) ----
    order_e = np.argsort(r, kind="stable")
    r_s = r[order_e]
    starts = np.zeros(N_NODES, dtype=np.int64)
    np.cumsum(deg[:-1], out=starts[1:])
    k = np.arange(E, dtype=np.int64) - starts[r_s]  # rank within receiver

    ea16 = np.ascontiguousarray(edge_attr, dtype=np.float32).astype(F16)[order_e]
    ecore = core_of[r_s]
    esg = sg_of[r_s]
    ej = j_of[r_s]
    ecol = colw[r_s]

    dense = k < KD
    TA = np.zeros((N_CORES, NSG, QD, 4, SG, D), dtype=F16)
    TA[ecore[dense], esg[dense], k[dense] // 4, k[dense] % 4,
       (ej * P + ecol)[dense]] = ea16[dense]
    # partitions (slot-in-pass, feat), free (pass, node)
    arr_A = np.ascontiguousarray(TA.transpose(0, 1, 3, 5, 2, 4)).reshape(
        N_CORES, NSG, P, A_ELEMS
    )

    # overflow: bucket per window, tiles of 128 edges
    ov = ~dense
    ovi = np.nonzero(ov)[0]
    wkey = win[r_s[ovi]]
    cnt = np.bincount(wkey, minlength=NW)
    m_l = max(1, int(-(-cnt.max() // P)))
    NT = WPS * m_l
    starts2 = np.zeros(NW, dtype=np.int64)
    np.cumsum(cnt[:-1], out=starts2[1:])
    o2 = np.argsort(wkey, kind="stable")
    ovs = ovi[o2]  # overflow edges grouped by window
    slot2 = np.arange(len(ovs), dtype=np.int64) - starts2[wkey[o2]]
    t2 = slot2 // P
    p2 = slot2 % P

    arr_B = np.zeros((N_CORES, NSG, P, NT, D), dtype=F16)
    arr_B[ecore[ovs], esg[ovs], p2, ej[ovs] * m_l + t2] = ea16[ovs]

    widx = np.full((NW, m_l * P), -1.0, dtype=np.float32)
    widx[wkey[o2], slot2] = ecol[ovs].astype(np.float32)
    # (core, sg, j, t, p) -> (core, p, sg, j, t)
    idx_arr = np.ascontiguousarray(
        widx.reshape(N_CORES, NSG, WPS, m_l, P).transpose(0, 4, 1, 2, 3)
    ).reshape(N_CORES, P, NSG * NT).astype(F16)

    edges_in = np.concatenate(
        [arr_A, arr_B.reshape(N_CORES, NSG, P, NT * D)], axis=3
    )

    ident4 = np.ascontiguousarray(np.tile(np.eye(D, dtype=F16), (4, 1)))

    nodeC = np.zeros((N_CORES, NPC_PAD, D), dtype=F16)
    nodeC[core_of, loc_of] = node_attr.astype(np.float32).astype(F16)
    nodeT = np.ascontiguousarray(nodeC.transpose(0, 2, 1))  # [8, 32, 12800]

    g0 = global_attr.astype(np.float32).reshape(1, D)
    W1 = W1.astype(np.float32)
    b1p = (b1.astype(np.float32) + (g0 @ W1[2 * D:]).reshape(-1)).reshape(D, 1)
    w1c = np.ascontiguousarray(W1[: 2 * D].astype(F16))  # [64, 32] node|agg
    w2 = np.ascontiguousarray(W2.astype(np.float32).astype(F16))
    b2 = b2.astype(np.float32).reshape(D, 1)

    in_maps = []
    for c in range(N_CORES):
        in_maps.append(
            {
                "edges": edges_in[c],
                "idx": idx_arr[c],
                "ident4": ident4,
                "nodeT": nodeT[c],
                "w1c": w1c,
                "w2": w2,
                "b1p": b1p,
                "b2": b2,
            }
        )
    return in_maps, m_l, core_of, loc_of


def _build_program(m_l):
    if m_l in _prog_cache:
        return _prog_cache[m_l]

    f32 = mybir.dt.float32
    f16 = mybir.dt.float16
    nc = bacc.Bacc(
        "TRN2", target_bir_lowering=False, debug=False, num_devices=N_CORES
    )

    NT = WPS * m_l
    B_ELEMS = NT * D

    edges_d = nc.dram_tensor(
        "edges", [NSG, P, A_ELEMS + B_ELEMS], f16, kind="ExternalInput"
    )
    idx_d = nc.dram_tensor("idx", [P, NSG * NT], f16, kind="ExternalInput")
    ident4_d = nc.dram_tensor("ident4", [P, D], f16, kind="ExternalInput")
    nodeT_d = nc.dram_tensor("nodeT", [D, NPC_PAD], f16, kind="ExternalInput")
    w1c_d = nc.dram_tensor("w1c", [2 * D, D], f16, kind="ExternalInput")
    w2_d = nc.dram_tensor("w2", [D, D], f16, kind="ExternalInput")
    b1p_d = nc.dram_tensor("b1p", [D, 1], f32, kind="ExternalInput")
    b2_d = nc.dram_tensor("b2", [D, 1], f32, kind="ExternalInput")
    outT_d = nc.dram_tensor("outT", [D, NPC_PAD], f16, kind="ExternalOutput")

    with tile.TileContext(nc) as tc:
        with (
            tc.tile_pool(name="const", bufs=1) as cpool,
            tc.tile_pool(name="edges", bufs=4) as epool,
            tc.tile_pool(name="oh", bufs=3) as opool,
            tc.tile_pool(name="mlp", bufs=3) as mpool,
            tc.tile_pool(name="psA", bufs=3, space="PSUM") as pspool,
            tc.tile_pool(name="psM", bufs=2, space="PSUM") as pmpool,
        ):
            iota32 = cpool.tile([P, NT, P], mybir.dt.int32)
            nc.gpsimd.iota(
                iota32[:], pattern=[[0, NT], [1, P]], base=0, channel_multiplier=0
            )
            iotab = cpool.tile([P, NT, P], f16)
            nc.vector.tensor_copy(out=iotab[:], in_=iota32[:])

            ident4_sb = cpool.tile([P, D], f16)
            nc.sync.dma_start(out=ident4_sb[:], in_=ident4_d.ap())
            idx_all = cpool.tile([P, NSG * NT], f16)
            nc.sync.dma_start(out=idx_all[:], in_=idx_d.ap())
            w1c_sb = cpool.tile([2 * D, D], f16)
            nc.sync.dma_start(out=w1c_sb[:], in_=w1c_d.ap())
            w2_sb = cpool.tile([D, D], f16)
            nc.sync.dma_start(out=w2_sb[:], in_=w2_d.ap())
            b1p_sb = cpool.tile([D, 1], f32)
            nc.sync.dma_start(out=b1p_sb[:], in_=b1p_d.ap())
            b2_sb = cpool.tile([D, 1], f32)
            nc.sync.dma_start(out=b2_sb[:], in_=b2_d.ap())

            for s in range(NSG):
                cols = slice(s * SG, (s + 1) * SG)

                edges_t = epool.tile([P, A_ELEMS + B_ELEMS], f16)
                nc.sync.dma_start(out=edges_t[:], in_=edges_d.ap()[s])
                oh = opool.tile([P, NT, P], f16)
                nc.vector.tensor_tensor(
                    out=oh[:],
                    in0=iotab[:],
                    in1=idx_all[:, s * NT : (s + 1) * NT].to_broadcast([P, NT, P]),
                    op=mybir.AluOpType.is_equal,
                )
                ps = pspool.tile([D, SG], f32)
                for q in range(QD):
                    nc.tensor.matmul(
                        out=ps[:],
                        lhsT=ident4_sb[:],
                        rhs=edges_t[:, q * SG : (q + 1) * SG],
                        start=(q == 0),
                        stop=False,
                        skip_group_check=True,
                    )
                for jt in range(NT):
                    jj = jt // m_l
                    base = A_ELEMS + jt * D
                    nc.tensor.matmul(
                        out=ps[:, jj * P : (jj + 1) * P],
                        lhsT=edges_t[:, base : base + D],
                        rhs=oh[:, jt, :],
                        start=False,
                        stop=(jt == NT - 1),
                        skip_group_check=True,
                    )

                comb = mpool.tile([2 * D, SG], f16, tag="comb")
                nc.sync.dma_start(out=comb[:D, :], in_=nodeT_d.ap()[:, cols])
                nc.scalar.activation(
                    out=comb[D:, :],
                    in_=ps[:],
                    func=mybir.ActivationFunctionType.Copy,
                )
                ph = pmpool.tile([D, SG], f32, tag="ph")
                nc.tensor.matmul(
                    out=ph[:], lhsT=w1c_sb[:], rhs=comb[:], start=True, stop=True
                )
                hH = mpool.tile([D, SG], f16, tag="hH")
                nc.scalar.activation(
                    out=hH[:],
                    in_=ph[:],
                    func=mybir.ActivationFunctionType.Relu,
                    bias=b1p_sb[:],
                    scale=1.0,
                )
                po = pmpool.tile([D, SG], f32, tag="po")
                nc.tensor.matmul(
                    out=po[:], lhsT=w2_sb[:], rhs=hH[:], start=True, stop=True
                )
                ot = mpool.tile([D, SG], f16, tag="ot")
                nc.vector.tensor_tensor(
                    out=ot[:],
                    in0=po[:],
                    in1=b2_sb[:].to_broadcast([D, SG]),
                    op=mybir.AluOpType.add,
                )
                nc.sync.dma_start(out=outT_d.ap()[:, cols], in_=ot[:])

    nc.finalize()
    _prog_cache[m_l] = nc
    return nc


def kernel(**inputs):
    in_maps, m_l, core_of, loc_of = _host_prep(**inputs)
    nc = _build_program(m_l)
    trace = bool(os.environ.get("KERNEL_TRACE"))
    res = run_bass_kernel_spmd(nc, in_maps, list(range(N_CORES)), trace=trace)
    if trace:
        print(f"HW exec time: {res.exec_time_ns} ns")
        print(f"mean exec time: {res.mean_exec_time_ns} ns")
    big = np.stack([res.results[c]["outT"] for c in range(N_CORES)])  # [8,32,12800]
    out = big.transpose(0, 2, 1)[core_of, loc_of].astype(np.float32)
    return out


# revision 3
# speedup vs baseline: 2.2687x; 1.0250x over previous
"""Trainium2 Bass kernel for GNN NodeBlock (segment_sum + MLP), 8-core SPMD.

Strategy (node-sharded, fp16, degree-balanced):
  - Host assigns nodes to (core, supergroup, window, col) by dealing them
    round-robin in decreasing order of overflow degree, so every 128-node
    window sees ~equal overflow (max <= 256 edges -> m_l=2) and every core
    ~equal edge bytes. Output is un-permuted on the host.
  - All tensor data travels as fp16 (the 2e-2 error gate leaves ~40x
    margin; measured pipeline rel err ~5e-4), halving HBM bytes and PE
    work vs an f32-exact hi/lo split.
  - Dense path: 16 fp16 slots per node, 4 slots stacked on partitions
    (4 slots x 32 feats = 128), summed into PSUM by a [I32;I32;I32;I32]
    stationary: 4 matmuls of 512 cols per supergroup.
  - Overflow path (edges past a node's 16th): one-hot matmul per
    128-edge tile (is_equal vs iota on DVE), 2 tiles per window.
  - MLP: agg is drained once to fp16, combined with a per-supergroup
    node-attr DMA into a [64, 512] tile; W1 (globals folded into b1 on
    host), ReLU via scalar ACT bias path, W2, +b2 via DVE; fp16 out.
  - No collectives: cores own disjoint node ranges; host gathers.
"""

import os

import numpy as np

import concourse.bacc as bacc
import concourse.bass as bass
import concourse.mybir as mybir
import concourse.tile as tile
from concourse.bass_utils import run_bass_kernel_spmd

F16 = np.float16

N_NODES = 100000
N_CORES = 8
P = 128
SG = 512  # nodes per supergroup (4 windows of 128)
WPS = SG // P  # 4 windows per supergroup
NSG = 25  # supergroups per core
NPC_PAD = NSG * SG  # 12800 node columns per core (125/128 used per window)
NW = N_CORES * NSG * WPS  # 800 windows
KD = 16  # dense slots per node
QD = KD // 4  # 4 slots per matmul pass -> 4 dense matmuls per supergroup
D = 32
A_ELEMS = QD * SG  # 2048 dense cols per supergroup

_prog_cache = {}


def _host_prep(node_attr, edge_index, edge_attr, global_attr, W1, b1, W2, b2):
    E = edge_attr.shape[0]
    r = np.ascontiguousarray(edge_index[1]).astype(np.int64)

    # ---- balanced node placement ----
    deg = np.bincount(r, minlength=N_NODES)
    excess = np.maximum(deg - KD, 0)
    order_n = np.argsort(-excess, kind="stable")
    win = np.empty(N_NODES, np.int64)
    win[order_n] = np.arange(N_NODES) % NW
    colw = np.empty(N_NODES, np.int64)
    colw[order_n] = np.arange(N_NODES) // NW  # 0..124
    core_of = win // (NSG * WPS)
    sg_of = (win % (NSG * WPS)) // WPS
    j_of = win % WPS
    loc_of = sg_of * SG + j_of * P + colw  # column within core

    # ---- per-edge placement (receiver-sorted) ----
    order_e = np.argsort(r, kind="stable")
    r_s = r[order_e]
    starts = np.zeros(N_NODES, dtype=np.int64)
    np.cumsum(deg[:-1], out=starts[1:])
    k = np.arange(E, dtype=np.int64) - starts[r_s]  # rank within receiver

    ea16 = np.ascontiguousarray(edge_attr, dtype=np.float32).astype(F16)[order_e]
    ecore = core_of[r_s]
    esg = sg_of[r_s]
    ej = j_of[r_s]
    ecol = colw[r_s]

    dense = k < KD
    TA = np.zeros((N_CORES, NSG, QD, 4, SG, D), dtype=F16)
    TA[ecore[dense], esg[dense], k[dense] // 4, k[dense] % 4,
       (ej * P + ecol)[dense]] = ea16[dense]
    # partitions (slot-in-pass, feat), free (pass, node)
    arr_A = np.ascontiguousarray(TA.transpose(0, 1, 3, 5, 2, 4)).reshape(
        N_CORES, NSG, P, A_ELEMS
    )

    # overflow: bucket per window, tiles of 128 edges
    ov = ~dense
    ovi = np.nonzero(ov)[0]
    wkey = win[r_s[ovi]]
    cnt = np.bincount(wkey, minlength=NW)
    m_l = max(1, int(-(-cnt.max() // P)))
    NT = WPS * m_l
    starts2 = np.zeros(NW, dtype=np.int64)
    np.cumsum(cnt[:-1], out=starts2[1:])
    o2 = np.argsort(wkey, kind="stable")
    ovs = ovi[o2]  # overflow edges grouped by window
    slot2 = np.arange(len(ovs), dtype=np.int64) - starts2[wkey[o2]]
    t2 = slot2 // P
    p2 = slot2 % P

    arr_B = np.zeros((N_CORES, NSG, P, NT, D), dtype=F16)
    arr_B[ecore[ovs], esg[ovs], p2, ej[ovs] * m_l + t2] = ea16[ovs]

    widx = np.full((NW, m_l * P), -1.0, dtype=np.float32)
    widx[wkey[o2], slot2] = ecol[ovs].astype(np.float32)
    # (core, sg, j, t, p) -> (core, p, sg, j, t)
    idx_arr = np.ascontiguousarray(
        widx.reshape(N_CORES, NSG, WPS, m_l, P).transpose(0, 4, 1, 2, 3)
    ).reshape(N_CORES, P, NSG * NT).astype(F16)

    edges_in = np.concatenate(
        [arr_A, arr_B.reshape(N_CORES, NSG, P, NT * D)], axis=3
    )

    ident4 = np.ascontiguousarray(np.tile(np.eye(D, dtype=F16), (4, 1)))

    nodeC = np.zeros((N_CORES, NPC_PAD, D), dtype=F16)
    nodeC[core_of, loc_of] = node_attr.astype(np.float32).astype(F16)
    nodeT = np.ascontiguousarray(nodeC.transpose(0, 2, 1))  # [8, 32, 12800]

    g0 = global_attr.astype(np.float32).reshape(1, D)
    W1 = W1.astype(np.float32)
    b1p = (b1.astype(np.float32) + (g0 @ W1[2 * D:]).reshape(-1)).reshape(D, 1)
    w1c = np.ascontiguousarray(W1[: 2 * D].astype(F16))  # [64, 32] node|agg
    w2 = np.ascontiguousarray(W2.astype(np.float32).astype(F16))
    b2 = b2.astype(np.float32).reshape(D, 1)

    in_maps = []
    for c in range(N_CORES):
        in_maps.append(
            {
                "edges": edges_in[c],
                "idx": idx_arr[c],
                "ident4": ident4,
                "nodeT": nodeT[c],
                "w1c": w1c,
                "w2": w2,
                "b1p": b1p,
                "b2": b2,
            }
        )
    return in_maps, m_l, core_of, loc_of


def _build_program(m_l):
    if m_l in _prog_cache:
        return _prog_cache[m_l]

    f32 = mybir.dt.float32
    f16 = mybir.dt.float16
    nc = bacc.Bacc(
        "TRN2", target_bir_lowering=False, debug=False, num_devices=N_CORES
    )

    NT = WPS * m_l
    B_ELEMS = NT * D

    edges_d = nc.dram_tensor(
        "edges", [NSG, P, A_ELEMS + B_ELEMS], f16, kind="ExternalInput"
    )
    idx_d = nc.dram_tensor("idx", [P, NSG * NT], f16, kind="ExternalInput")
    ident4_d = nc.dram_tensor("ident4", [P, D], f16, kind="ExternalInput")
    nodeT_d = nc.dram_tensor("nodeT", [D, NPC_PAD], f16, kind="ExternalInput")
    w1c_d = nc.dram_tensor("w1c", [2 * D, D], f16, kind="ExternalInput")
    w2_d = nc.dram_tensor("w2", [D, D], f16, kind="ExternalInput")
    b1p_d = nc.dram_tensor("b1p", [D, 1], f32, kind="ExternalInput")
    b2_d = nc.dram_tensor("b2", [D, 1], f32, kind="ExternalInput")
    outT_d = nc.dram_tensor("outT", [D, NPC_PAD], f16, kind="ExternalOutput")

    with tile.TileContext(nc) as tc:
        with (
            tc.tile_pool(name="const", bufs=1) as cpool,
            tc.tile_pool(name="edges", bufs=4) as epool,
            tc.tile_pool(name="oh", bufs=3) as opool,
            tc.tile_pool(name="mlp", bufs=3) as mpool,
            tc.tile_pool(name="psA", bufs=2, space="PSUM") as pspool,
            tc.tile_pool(name="psM", bufs=2, space="PSUM") as pmpool,
        ):
            iota32 = cpool.tile([P, NT, P], mybir.dt.int32)
            nc.gpsimd.iota(
                iota32[:], pattern=[[0, NT], [1, P]], base=0, channel_multiplier=0
            )
            iotab = cpool.tile([P, NT, P], f16)
            nc.vector.tensor_copy(out=iotab[:], in_=iota32[:])

            ident4_sb = cpool.tile([P, D], f16)
            nc.sync.dma_start(out=ident4_sb[:], in_=ident4_d.ap())
            idx_all = cpool.tile([P, NSG * NT], f16)
            nc.sync.dma_start(out=idx_all[:], in_=idx_d.ap())
            node_sb = cpool.tile([D, NPC_PAD], f16)
            nc.sync.dma_start(out=node_sb[:], in_=nodeT_d.ap())
            w1n_sb = cpool.tile([D, D], f16)
            nc.sync.dma_start(out=w1n_sb[:], in_=w1c_d.ap()[:D])
            w1a_sb = cpool.tile([D, D], f16)
            nc.sync.dma_start(out=w1a_sb[:], in_=w1c_d.ap()[D:])
            w2_sb = cpool.tile([D, D], f16)
            nc.sync.dma_start(out=w2_sb[:], in_=w2_d.ap())
            b1p_sb = cpool.tile([D, 1], f32)
            nc.sync.dma_start(out=b1p_sb[:], in_=b1p_d.ap())
            b2_sb = cpool.tile([D, 1], f32)
            nc.sync.dma_start(out=b2_sb[:], in_=b2_d.ap())

            # software-pipelined: iter s runs agg(s), W1(s-1), W2+out(s-2)
            aggs = [None] * NSG  # fp16 aggregate [D, SG] per supergroup
            phs = [None] * NSG  # PSUM W1 output
            hHs = [None] * NSG  # fp16 relu output
            ohs = [None] * NSG  # one-hot tiles

            def build_oh(s):
                oh = opool.tile([P, NT, P], f16)
                nc.vector.tensor_tensor(
                    out=oh[:],
                    in0=iotab[:],
                    in1=idx_all[:, s * NT : (s + 1) * NT].to_broadcast([P, NT, P]),
                    op=mybir.AluOpType.is_equal,
                )
                ohs[s] = oh

            build_oh(0)
            for s in range(NSG + 2):
                if s < NSG:
                    if s + 1 < NSG:
                        build_oh(s + 1)
                    edges_t = epool.tile([P, A_ELEMS + B_ELEMS], f16)
                    nc.sync.dma_start(out=edges_t[:], in_=edges_d.ap()[s])
                    ps = pspool.tile([D, SG], f32)
                    for q in range(QD):
                        nc.tensor.matmul(
                            out=ps[:],
                            lhsT=ident4_sb[:],
                            rhs=edges_t[:, q * SG : (q + 1) * SG],
                            start=(q == 0),
                            stop=False,
                            skip_group_check=True,
                        )
                    for jt in range(NT):
                        jj = jt // m_l
                        base = A_ELEMS + jt * D
                        nc.tensor.matmul(
                            out=ps[:, jj * P : (jj + 1) * P],
                            lhsT=edges_t[:, base : base + D],
                            rhs=ohs[s][:, jt, :],
                            start=False,
                            stop=(jt == NT - 1),
                            skip_group_check=True,
                        )
                    agg = mpool.tile([D, SG], f16, tag="agg")
                    nc.scalar.activation(
                        out=agg[:],
                        in_=ps[:],
                        func=mybir.ActivationFunctionType.Copy,
                    )
                    aggs[s] = agg

                if 0 <= s - 1 < NSG:
                    p = s - 1
                    cols = slice(p * SG, (p + 1) * SG)
                    ph = pmpool.tile([D, SG], f32, tag="ph")
                    nc.tensor.matmul(
                        out=ph[:],
                        lhsT=w1n_sb[:],
                        rhs=node_sb[:, cols],
                        start=True,
                        stop=False,
                        skip_group_check=True,
                    )
                    nc.tensor.matmul(
                        out=ph[:],
                        lhsT=w1a_sb[:],
                        rhs=aggs[p][:],
                        start=False,
                        stop=True,
                        skip_group_check=True,
                    )
                    hH = mpool.tile([D, SG], f16, tag="hH")
                    nc.scalar.activation(
                        out=hH[:],
                        in_=ph[:],
                        func=mybir.ActivationFunctionType.Relu,
                        bias=b1p_sb[:],
                        scale=1.0,
                    )
                    phs[p] = ph
                    hHs[p] = hH

                if 0 <= s - 2 < NSG:
                    p = s - 2
                    cols = slice(p * SG, (p + 1) * SG)
                    po = pmpool.tile([D, SG], f32, tag="po")
                    nc.tensor.matmul(
                        out=po[:],
                        lhsT=w2_sb[:],
                        rhs=hHs[p][:],
                        start=True,
                        stop=True,
                        skip_group_check=True,
                    )
                    ot = mpool.tile([D, SG], f16, tag="ot")
                    nc.vector.tensor_tensor(
                        out=ot[:],
                        in0=po[:],
                        in1=b2_sb[:].to_broadcast([D, SG]),
                        op=mybir.AluOpType.add,
                    )
                    nc.sync.dma_start(out=outT_d.ap()[:, cols], in_=ot[:])

    nc.finalize()
    _prog_cache[m_l] = nc
    return nc


def kernel(**inputs):
    in_maps, m_l, core_of, loc_of = _host_prep(**inputs)
    nc = _build_program(m_l)
    trace = bool(os.environ.get("KERNEL_TRACE"))
    res = run_bass_kernel_spmd(nc, in_maps, list(range(N_CORES)), trace=trace)
    if trace:
        print(f"HW exec time: {res.exec_time_ns} ns")
        print(f"mean exec time: {res.mean_exec_time_ns} ns")
    big = np.stack([res.results[c]["outT"] for c in range(N_CORES)])  # [8,32,12800]
    out = big.transpose(0, 2, 1)[core_of, loc_of].astype(np.float32)
    return out


# revision 5
# speedup vs baseline: 2.7697x; 1.2208x over previous
"""Trainium2 Bass kernel for GNN NodeBlock (segment_sum + MLP), 8-core SPMD.

Strategy (node-sharded, fp16, degree-balanced):
  - Host assigns nodes to (core, supergroup, window, col) by dealing them
    round-robin in decreasing order of overflow degree, so every 128-node
    window sees ~equal overflow (max <= 256 edges -> m_l=2) and every core
    ~equal edge bytes. Output is un-permuted on the host.
  - All tensor data travels as fp16 (the 2e-2 error gate leaves ~40x
    margin; measured pipeline rel err ~5e-4), halving HBM bytes and PE
    work vs an f32-exact hi/lo split.
  - Dense path: 16 fp16 slots per node, 4 slots stacked on partitions
    (4 slots x 32 feats = 128), summed into PSUM by a [I32;I32;I32;I32]
    stationary: 4 matmuls of 512 cols per supergroup.
  - Overflow path (edges past a node's 16th): one-hot matmul per
    128-edge tile (is_equal vs iota on DVE), 2 tiles per window.
  - MLP: agg is drained once to fp16, combined with a per-supergroup
    node-attr DMA into a [64, 512] tile; W1 (globals folded into b1 on
    host), ReLU via scalar ACT bias path, W2, +b2 via DVE; fp16 out.
  - No collectives: cores own disjoint node ranges; host gathers.
"""

import os

import numpy as np

import concourse.bacc as bacc
import concourse.bass as bass
import concourse.mybir as mybir
import concourse.tile as tile
from concourse.bass_utils import run_bass_kernel_spmd

F16 = np.float16

N_NODES = 100000
N_CORES = 8
P = 128
SG = 512  # nodes per supergroup (4 windows of 128)
WPS = SG // P  # 4 windows per supergroup
NSG = 25  # supergroups per core
NPC_PAD = NSG * SG  # 12800 node columns per core (125/128 used per window)
NW = N_CORES * NSG * WPS  # 800 windows
KD = 16  # dense slots per node
QD = KD // 4  # 4 slots per matmul pass -> 4 dense matmuls per supergroup
D = 32
A_ELEMS = QD * SG  # 2048 dense cols per supergroup

_prog_cache = {}


def _host_prep(node_attr, edge_index, edge_attr, global_attr, W1, b1, W2, b2):
    E = edge_attr.shape[0]
    r = np.ascontiguousarray(edge_index[1]).astype(np.int64)

    # ---- balanced node placement ----
    deg = np.bincount(r, minlength=N_NODES)
    excess = np.maximum(deg - KD, 0)
    order_n = np.argsort(-excess, kind="stable")
    win = np.empty(N_NODES, np.int64)
    win[order_n] = np.arange(N_NODES) % NW
    colw = np.empty(N_NODES, np.int64)
    colw[order_n] = np.arange(N_NODES) // NW  # 0..124
    core_of = win // (NSG * WPS)
    sg_of = (win % (NSG * WPS)) // WPS
    j_of = win % WPS
    loc_of = sg_of * SG + j_of * P + colw  # column within core

    # ---- per-edge placement (receiver-sorted) ----
    order_e = np.argsort(r, kind="stable")
    r_s = r[order_e]
    starts = np.zeros(N_NODES, dtype=np.int64)
    np.cumsum(deg[:-1], out=starts[1:])
    k = np.arange(E, dtype=np.int64) - starts[r_s]  # rank within receiver

    ea16 = np.ascontiguousarray(edge_attr, dtype=np.float32).astype(F16)[order_e]
    ecore = core_of[r_s]
    esg = sg_of[r_s]
    ej = j_of[r_s]
    ecol = colw[r_s]

    dense = k < KD
    TA = np.zeros((N_CORES, NSG, QD, 4, SG, D), dtype=F16)
    TA[ecore[dense], esg[dense], k[dense] // 4, k[dense] % 4,
       (ej * P + ecol)[dense]] = ea16[dense]
    # partitions (slot-in-pass, feat), free (pass, node)
    arr_A = np.ascontiguousarray(TA.transpose(0, 1, 3, 5, 2, 4)).reshape(
        N_CORES, NSG, P, A_ELEMS
    )

    # overflow: bucket per window, tiles of 128 edges
    ov = ~dense
    ovi = np.nonzero(ov)[0]
    wkey = win[r_s[ovi]]
    cnt = np.bincount(wkey, minlength=NW)
    m_l = max(1, int(-(-cnt.max() // P)))
    NT = WPS * m_l
    starts2 = np.zeros(NW, dtype=np.int64)
    np.cumsum(cnt[:-1], out=starts2[1:])
    o2 = np.argsort(wkey, kind="stable")
    ovs = ovi[o2]  # overflow edges grouped by window
    slot2 = np.arange(len(ovs), dtype=np.int64) - starts2[wkey[o2]]
    t2 = slot2 // P
    p2 = slot2 % P

    arr_B = np.zeros((N_CORES, NSG, P, NT, D), dtype=F16)
    arr_B[ecore[ovs], esg[ovs], p2, ej[ovs] * m_l + t2] = ea16[ovs]

    widx = np.full((NW, m_l * P), -1.0, dtype=np.float32)
    widx[wkey[o2], slot2] = ecol[ovs].astype(np.float32)
    # (core, sg, j, t, p) -> (core, p, sg, j, t)
    idx_arr = np.ascontiguousarray(
        widx.reshape(N_CORES, NSG, WPS, m_l, P).transpose(0, 4, 1, 2, 3)
    ).reshape(N_CORES, P, NSG * NT).astype(F16)

    edges_in = np.concatenate(
        [arr_A, arr_B.reshape(N_CORES, NSG, P, NT * D)], axis=3
    )

    ident4 = np.ascontiguousarray(np.tile(np.eye(D, dtype=F16), (4, 1)))

    nodeC = np.zeros((N_CORES, NPC_PAD, D), dtype=F16)
    nodeC[core_of, loc_of] = node_attr.astype(np.float32).astype(F16)
    nodeT = np.ascontiguousarray(nodeC.transpose(0, 2, 1))  # [8, 32, 12800]

    g0 = global_attr.astype(np.float32).reshape(1, D)
    W1 = W1.astype(np.float32)
    b1p = (b1.astype(np.float32) + (g0 @ W1[2 * D:]).reshape(-1)).reshape(D, 1)
    w1c = np.ascontiguousarray(W1[: 2 * D].astype(F16))  # [64, 32] node|agg
    w2 = np.ascontiguousarray(W2.astype(np.float32).astype(F16))
    b2 = b2.astype(np.float32).reshape(D, 1)

    in_maps = []
    for c in range(N_CORES):
        in_maps.append(
            {
                "edges": edges_in[c],
                "idx": idx_arr[c],
                "ident4": ident4,
                "nodeT": nodeT[c],
                "w1c": w1c,
                "w2": w2,
                "b1p": b1p,
                "b2": b2,
            }
        )
    return in_maps, m_l, core_of, loc_of


def _build_program(m_l):
    if m_l in _prog_cache:
        return _prog_cache[m_l]

    f32 = mybir.dt.float32
    f16 = mybir.dt.float16
    nc = bacc.Bacc(
        "TRN2", target_bir_lowering=False, debug=False, num_devices=N_CORES
    )

    NT = WPS * m_l
    B_ELEMS = NT * D

    edges_d = nc.dram_tensor(
        "edges", [NSG, P, A_ELEMS + B_ELEMS], f16, kind="ExternalInput"
    )
    idx_d = nc.dram_tensor("idx", [P, NSG * NT], f16, kind="ExternalInput")
    ident4_d = nc.dram_tensor("ident4", [P, D], f16, kind="ExternalInput")
    nodeT_d = nc.dram_tensor("nodeT", [D, NPC_PAD], f16, kind="ExternalInput")
    w1c_d = nc.dram_tensor("w1c", [2 * D, D], f16, kind="ExternalInput")
    w2_d = nc.dram_tensor("w2", [D, D], f16, kind="ExternalInput")
    b1p_d = nc.dram_tensor("b1p", [D, 1], f32, kind="ExternalInput")
    b2_d = nc.dram_tensor("b2", [D, 1], f32, kind="ExternalInput")
    outT_d = nc.dram_tensor("outT", [D, NPC_PAD], f16, kind="ExternalOutput")

    with tile.TileContext(nc) as tc:
        with (
            tc.tile_pool(name="const", bufs=1) as cpool,
            tc.tile_pool(name="edges", bufs=6) as epool,
            tc.tile_pool(name="oh", bufs=3) as opool,
            tc.tile_pool(name="mlp", bufs=3) as mpool,
            tc.tile_pool(name="psA", bufs=3, space="PSUM") as pspool,
            tc.tile_pool(name="psM", bufs=2, space="PSUM") as pmpool,
        ):
            iota32 = cpool.tile([P, NT, P], mybir.dt.int32)
            nc.gpsimd.iota(
                iota32[:], pattern=[[0, NT], [1, P]], base=0, channel_multiplier=0
            )
            iotab = cpool.tile([P, NT, P], f16)
            nc.vector.tensor_copy(out=iotab[:], in_=iota32[:])

            ident4_sb = cpool.tile([P, D], f16)
            nc.sync.dma_start(out=ident4_sb[:], in_=ident4_d.ap())
            idx_all = cpool.tile([P, NSG * NT], f16)
            nc.sync.dma_start(out=idx_all[:], in_=idx_d.ap())
            node_sb = cpool.tile([D, NPC_PAD], f16)
            nc.sync.dma_start(out=node_sb[:], in_=nodeT_d.ap())
            w1n_sb = cpool.tile([D, D], f16)
            nc.sync.dma_start(out=w1n_sb[:], in_=w1c_d.ap()[:D])
            w1a_sb = cpool.tile([D, D], f16)
            nc.sync.dma_start(out=w1a_sb[:], in_=w1c_d.ap()[D:])
            w2_sb = cpool.tile([D, D], f16)
            nc.sync.dma_start(out=w2_sb[:], in_=w2_d.ap())
            b1p_sb = cpool.tile([D, 1], f32)
            nc.sync.dma_start(out=b1p_sb[:], in_=b1p_d.ap())
            b2_sb = cpool.tile([D, 1], f32)
            nc.sync.dma_start(out=b2_sb[:], in_=b2_d.ap())

            # software-pipelined: iter s runs agg(s), W1(s-1), W2+out(s-2)
            aggs = [None] * NSG  # fp16 aggregate [D, SG] per supergroup
            phs = [None] * NSG  # PSUM W1 output
            hHs = [None] * NSG  # fp16 relu output
            ohs = [None] * NSG  # one-hot tiles

            def build_oh(s):
                oh = opool.tile([P, NT, P], f16)
                nc.vector.tensor_tensor(
                    out=oh[:],
                    in0=iotab[:],
                    in1=idx_all[:, s * NT : (s + 1) * NT].to_broadcast([P, NT, P]),
                    op=mybir.AluOpType.is_equal,
                )
                ohs[s] = oh

            build_oh(0)
            for s in range(NSG + 2):
                if s < NSG:
                    if s + 1 < NSG:
                        build_oh(s + 1)
                    edges_t = epool.tile([P, A_ELEMS + B_ELEMS], f16)
                    half = (A_ELEMS + B_ELEMS) // 2
                    nc.sync.dma_start(
                        out=edges_t[:, :half], in_=edges_d.ap()[s][:, :half]
                    )
                    nc.gpsimd.dma_start(
                        out=edges_t[:, half:], in_=edges_d.ap()[s][:, half:]
                    )
                    ps = pspool.tile([D, SG], f32)
                    for q in range(QD):
                        nc.tensor.matmul(
                            out=ps[:],
                            lhsT=ident4_sb[:],
                            rhs=edges_t[:, q * SG : (q + 1) * SG],
                            start=(q == 0),
                            stop=False,
                            skip_group_check=True,
                        )
                    for jt in range(NT):
                        jj = jt // m_l
                        base = A_ELEMS + jt * D
                        nc.tensor.matmul(
                            out=ps[:, jj * P : (jj + 1) * P],
                            lhsT=edges_t[:, base : base + D],
                            rhs=ohs[s][:, jt, :],
                            start=False,
                            stop=(jt == NT - 1),
                            skip_group_check=True,
                        )
                    agg = mpool.tile([D, SG], f16, tag="agg")
                    nc.scalar.activation(
                        out=agg[:],
                        in_=ps[:],
                        func=mybir.ActivationFunctionType.Copy,
                    )
                    aggs[s] = agg

                if 0 <= s - 1 < NSG:
                    p = s - 1
                    cols = slice(p * SG, (p + 1) * SG)
                    ph = pmpool.tile([D, SG], f32, tag="ph")
                    nc.tensor.matmul(
                        out=ph[:],
                        lhsT=w1n_sb[:],
                        rhs=node_sb[:, cols],
                        start=True,
                        stop=False,
                        skip_group_check=True,
                    )
                    nc.tensor.matmul(
                        out=ph[:],
                        lhsT=w1a_sb[:],
                        rhs=aggs[p][:],
                        start=False,
                        stop=True,
                        skip_group_check=True,
                    )
                    hH = mpool.tile([D, SG], f16, tag="hH")
                    nc.scalar.activation(
                        out=hH[:],
                        in_=ph[:],
                        func=mybir.ActivationFunctionType.Relu,
                        bias=b1p_sb[:],
                        scale=1.0,
                    )
                    phs[p] = ph
                    hHs[p] = hH

                if 0 <= s - 2 < NSG:
                    p = s - 2
                    cols = slice(p * SG, (p + 1) * SG)
                    po = pmpool.tile([D, SG], f32, tag="po")
                    nc.tensor.matmul(
                        out=po[:],
                        lhsT=w2_sb[:],
                        rhs=hHs[p][:],
                        start=True,
                        stop=True,
                        skip_group_check=True,
                    )
                    ot = mpool.tile([D, SG], f16, tag="ot")
                    nc.vector.tensor_tensor(
                        out=ot[:],
                        in0=po[:],
                        in1=b2_sb[:].to_broadcast([D, SG]),
                        op=mybir.AluOpType.add,
                    )
                    nc.sync.dma_start(out=outT_d.ap()[:, cols], in_=ot[:])

    nc.finalize()
    _prog_cache[m_l] = nc
    return nc


def kernel(**inputs):
    in_maps, m_l, core_of, loc_of = _host_prep(**inputs)
    nc = _build_program(m_l)
    trace = bool(os.environ.get("KERNEL_TRACE"))
    res = run_bass_kernel_spmd(nc, in_maps, list(range(N_CORES)), trace=trace)
    if trace:
        print(f"HW exec time: {res.exec_time_ns} ns")
        print(f"mean exec time: {res.mean_exec_time_ns} ns")
    big = np.stack([res.results[c]["outT"] for c in range(N_CORES)])  # [8,32,12800]
    out = big.transpose(0, 2, 1)[core_of, loc_of].astype(np.float32)
    return out
